# revision 38
# baseline (speedup 1.0000x reference)
"""Trainium2 Bass kernel for nn_CTCPerSpeakerExtractorConcatNNG (v3).

Sharding: 8 cores = (batch b, T-half th). Each core computes the shared
X/KV/K/V once for its T-half (+halo) and both speaker streams' attention
+ FFN for its 768 query rows. No collectives; host scatters/gathers.

v3 focus: keep the PE continuously busy (TRN2 DVFS only reaches 2.4GHz
after ~3us of gapless execution) and keep the ACT activation-table
resident (Sqrt/Gelu/Exp live in different table-sets; a switch costs
~1.3-2.7us and stalls dependent matmuls):
  - attention is software-pipelined: scores for tile lt+1 are emitted
    between/around the transpose+pV work of tile lt (additive band mask
    accumulated on the PE, denominators via the exp's accum_out).
    gpsimd turned out ~20x slower per element-op than DVE, so it only
    gets the FFN y3 bias adds and one F-phase add per tile.
  - LN rstd for the f/s LayerNorms uses a DVE Newton-iteration rsqrt
    (magic-constant seed + 2 iterations, ~5e-6 rel) so the Gelu table
    stays loaded through the whole FFN phase; Exp/Gelu tables are
    prefetched with dummy ops before their phase starts.
  - input DMAs are spread across the sync/scalar/gpsimd queues
    (~140GB/s each) and KVT/LNFT transposes are issued per-tile so
    dependent matmuls start as early as possible.
  - gpsimd takes the Xk gating, pms scaling and y3 bias adds; PSUM is
    rebalanced to 2x[X/yo/FFN] + 2x[psy] + 2x[scores] + 2x[transpose].

Key algebraic optimization (unchanged from v2): LN(X * gate) == LN(X)
for a positive per-row gate, so Q and the entire attention + Wo output
are SHARED by both speaker streams. LN gains/biases for kv/q/f folded
into the following matmuls on host; the x128 scaling of Wo/W1/W2/What
cancels in the downstream LayerNorms.
"""
import sys

for _p in ("/opt/trn_rl_repo", "/root/.axon_site/_ro/trn_rl_repo"):
    if _p not in sys.path:
        sys.path.append(_p)

from contextlib import ExitStack

import numpy as np
import ml_dtypes

import concourse.bass as bass
import concourse.bacc as bacc
import concourse.tile as tile
from concourse import mybir
from concourse.bass_utils import run_bass_kernel_spmd
from concourse.masks import make_identity

BF = mybir.dt.bfloat16
F32 = mybir.dt.float32
U32 = mybir.dt.uint32
AF = mybir.ActivationFunctionType
OP = mybir.AluOpType

B, T, D, KSP, H, BAND = 4, 1536, 512, 2, 8, 24
DH = D // H          # 64
P = 128
WIN = P + 2 * BAND   # 176
NC_D = D // P        # 4 chunks of contraction dim
DFF = 4 * D          # 2048
EPS = 1e-5

NQT = 6              # query tiles per stream (local tiles 1..6)
NSH = 8              # shared tiles (local rows [0, 1024))
TSH = NSH * P        # 1024
TQ = NQT * P         # 768
NV = 7               # V halo tiles at starts 104 + 128*j
SC = 128.0           # scale folded into Wo/W1/W2/What; cancels in LNs
MAGIC = float(np.uint32(0x5F3759DF).view(np.float32))


def _bcast_ap(dram_ap, parts=128):
    """[N] dram vector -> [parts, N] broadcast AP (partition step 0)."""
    return bass.AP(
        tensor=dram_ap.tensor,
        offset=dram_ap.offset,
        ap=[[0, parts]] + list(dram_ap.ap),
    )


def build_program(add_bo: bool, add_bin: bool = False,
                  add_bv: bool = False, sim_gelu_identity: bool = False) -> bass.Bass:
    # sim_gelu_identity: CoreSim has no Gelu; swap for Identity in sim runs.
    AF_GELU = AF.Identity if sim_gelu_identity else AF.Gelu
    nc = bacc.Bacc()

    # ---- DRAM I/O ----
    xmT = nc.dram_tensor("xmT", [D, TSH], BF, kind="ExternalInput")
    Wd = {}
    for nm, (di, do) in [("Win", (D, D)), ("Wq", (D, D)), ("Wk", (D, D)),
                         ("Wv", (D, D)), ("Wo", (D, D))]:
        Wd[nm] = nc.dram_tensor(nm, [di, do], BF, kind="ExternalInput")
    W1d = nc.dram_tensor("W1", [D, DFF], BF, kind="ExternalInput")
    W2d = nc.dram_tensor("W2", [DFF, D], BF, kind="ExternalInput")
    smalls_d = nc.dram_tensor("smalls", [P, 36], F32, kind="ExternalInput")
    rows_d = nc.dram_tensor("rows", [5, D], F32, kind="ExternalInput")
    masks_d = nc.dram_tensor("masks", [P, NQT * WIN], BF, kind="ExternalInput")
    out_d = nc.dram_tensor("out", [2 * TQ, D], F32, kind="ExternalOutput")
    out_t = out_d.rearrange("(n p) d -> n p d", p=P)

    with tile.TileContext(nc) as tc, ExitStack() as ctx:
        consts = ctx.enter_context(tc.tile_pool(name="consts", bufs=1))
        wpool = ctx.enter_context(tc.tile_pool(name="wpool", bufs=1))
        ktp = ctx.enter_context(tc.tile_pool(name="ktp", bufs=1))
        acts = ctx.enter_context(tc.tile_pool(name="acts", bufs=1))
        stream_p = ctx.enter_context(tc.tile_pool(name="stream_p", bufs=1))
        ln_nat_p = ctx.enter_context(tc.tile_pool(name="ln_nat_p", bufs=2))
        tT_p = ctx.enter_context(tc.tile_pool(name="tT_p", bufs=1))
        h1p = ctx.enter_context(tc.tile_pool(name="h1p", bufs=1))
        small = ctx.enter_context(tc.tile_pool(name="small", bufs=6))
        sm2 = ctx.enter_context(tc.tile_pool(name="sm2", bufs=3))
        outp = ctx.enter_context(tc.tile_pool(name="outp", bufs=3))
        psA = ctx.enter_context(tc.tile_pool(name="psA", bufs=2, space="PSUM"))
        psB = ctx.enter_context(tc.tile_pool(name="psB", bufs=2, space="PSUM"))
        psC = ctx.enter_context(tc.tile_pool(name="psC", bufs=2, space="PSUM"))

        # ---- constants / input DMAs (3 queues: sync, scalar, gpsimd) ----
        smalls = consts.tile([P, 36], F32, tag="smalls")
        nc.sync.dma_start(out=smalls, in_=smalls_d[:, :])
        xmT_s = ln_nat_p.tile([P, NC_D, TSH], BF, tag="ln_nat")
        xm_r = xmT.rearrange("(c p) t -> p c t", p=P)
        nc.sync.dma_start(out=xmT_s[:, :, 0:TSH // 2], in_=xm_r[:, :, 0:TSH // 2])
        nc.gpsimd.dma_start(out=xmT_s[:, :, TSH // 2:TSH],
                            in_=xm_r[:, :, TSH // 2:TSH])
        Ws = {}
        for nm in ("Win", "Wk", "Wv", "Wq", "Wo"):
            t = wpool.tile([P, NC_D, D], BF, tag=nm, name=f"W_{nm}")
            nc.scalar.dma_start(out=t, in_=Wd[nm].rearrange("(c p) o -> p c o", p=P))
            Ws[nm] = t
        W1s = wpool.tile([P, NC_D, DFF], BF, tag="W1")
        nc.scalar.dma_start(out=W1s, in_=W1d.rearrange("(c p) o -> p c o", p=P))
        W2s = wpool.tile([P, 16, D], BF, tag="W2")
        nc.scalar.dma_start(out=W2s, in_=W2d.rearrange("(c p) o -> p c o", p=P))

        ident = consts.tile([P, P], BF)
        make_identity(nc, ident)
        eps_t = consts.tile([P, 1], F32, tag="eps_t")
        nc.vector.memset(eps_t, EPS)
        magic = consts.tile([P, 12], F32, tag="magic")
        nc.vector.memset(magic, MAGIC)

        What = smalls[:, 0:12]
        bq4 = smalls[:, 12:16]
        bk4 = smalls[:, 16:20]
        b1_16 = smalls[:, 20:36]
        bin_b = consts.tile([P, D], F32, tag="bin_b")
        if add_bin:
            nc.sync.dma_start(out=bin_b, in_=_bcast_ap(rows_d[0, :]))

        def newton_rsqrt(var_ap, n, tag):
            """rstd[128, n] = 1/sqrt(var + eps) on DVE only (no ACT table)."""
            vpe = small.tile([P, 12], F32, tag=f"nv_{tag}", bufs=2)
            y = small.tile([P, 12], F32, tag=f"ny_{tag}", bufs=2)
            t = small.tile([P, 12], F32, tag=f"nt_{tag}", bufs=2)
            nc.vector.tensor_scalar(out=vpe[:, :n], in0=var_ap, scalar1=EPS,
                                    scalar2=None, op0=OP.add)
            nc.vector.tensor_scalar(out=y.bitcast(U32)[:, :n],
                                    in0=vpe.bitcast(U32)[:, :n], scalar1=1,
                                    scalar2=None, op0=OP.logical_shift_right)
            nc.vector.tensor_tensor(out=y.bitcast(U32)[:, :n],
                                    in0=magic.bitcast(U32)[:, :n],
                                    in1=y.bitcast(U32)[:, :n], op=OP.subtract)
            for _ in range(2):
                nc.vector.tensor_tensor(out=t[:, :n], in0=y[:, :n],
                                        in1=y[:, :n], op=OP.mult)
                nc.vector.tensor_tensor(out=t[:, :n], in0=t[:, :n],
                                        in1=vpe[:, :n], op=OP.mult)
                nc.vector.tensor_scalar(out=t[:, :n], in0=t[:, :n], scalar1=-0.5,
                                        scalar2=1.5, op0=OP.mult, op1=OP.add)
                nc.vector.tensor_tensor(out=y[:, :n], in0=y[:, :n],
                                        in1=t[:, :n], op=OP.mult)
            return y

        def ln_stats(in_ap, mv_ap):
            st = small.tile([P, 6], F32, tag="st6")
            nc.vector.bn_stats(out=st, in_=in_ap)
            nc.vector.bn_aggr(out=mv_ap, in_=st)

        # ---- A) X = xmT.T @ Win -> X_sb (psum freed fast); LN_kv; Xk ----
        lnkv = ln_nat_p.tile([P, NSH, D], BF, tag="ln_nat")
        KVT = tT_p.tile([P, NC_D, TSH], BF, tag="tT")
        Xk = acts.tile([P, 12, D], BF, tag="Xk")
        def a_tile(mt):
            ps = psA.tile([P, D], F32, tag="psA")
            for c in range(NC_D):
                nc.tensor.matmul(
                    ps, lhsT=xmT_s[:, c, mt * P:(mt + 1) * P], rhs=Ws["Win"][:, c, :],
                    start=(c == 0), stop=(c == NC_D - 1))
            if add_bin:
                psb = sm2.tile([P, D], F32, tag="Xpsb")
                nc.vector.tensor_tensor(out=psb, in0=ps, in1=bin_b, op=OP.add)
            else:
                psb = ps
            mv = small.tile([P, 2], F32, tag="mv")
            ln_stats(psb, mv)
            # ACT does ONLY Sqrt in this phase (Identity on ACT thrashes the
            # table set against Sqrt); lnkv is one DVE sub-mult that also
            # frees the X psum.
            sd = small.tile([P, 1], F32, tag="sd")
            nc.scalar.activation(out=sd, in_=mv[:, 1:2], func=AF.Sqrt, bias=eps_t)
            rstd = small.tile([P, 1], F32, tag="rstd")
            nc.vector.reciprocal(out=rstd, in_=sd)
            nc.vector.tensor_scalar(out=lnkv[:, mt, :], in0=psb,
                                    scalar1=mv[:, 0:1], scalar2=rstd,
                                    op0=OP.subtract, op1=OP.mult)
            teng = nc.sync if mt % 2 == 0 else nc.scalar
            teng.dma_start_transpose(
                out=KVT[:, :, mt * P:(mt + 1) * P], in_=lnkv[:, mt, :])
            if 1 <= mt <= NQT:
                # Xk = (sd*lnkv + m)*What' on gpsimd (consumed only in E)
                for k in range(2):
                    col = What[:, k * NQT + mt - 1:k * NQT + mt]
                    ab = small.tile([P, 2], F32, tag="ab")
                    nc.vector.tensor_scalar(out=ab[:, 0:1], in0=sd, scalar1=col,
                                            scalar2=None, op0=OP.mult)
                    nc.vector.tensor_scalar(out=ab[:, 1:2], in0=mv[:, 0:1],
                                            scalar1=col, scalar2=None, op0=OP.mult)
                    nc.gpsimd.tensor_scalar(
                        out=Xk[:, k * NQT + mt - 1, :], in0=lnkv[:, mt, :],
                        scalar1=ab[:, 0:1], scalar2=ab[:, 1:2],
                        op0=OP.mult, op1=OP.add)

        # remaining small consts (sync queue, after xmT half0 + transposes)
        masks = consts.tile([P, NQT, WIN], BF, tag="masks")
        nc.gpsimd.dma_start(out=masks, in_=masks_d.rearrange("p (n w) -> p n w", n=NQT))
        bv_b = consts.tile([P, D], F32, tag="bv_b")
        if add_bv:
            nc.gpsimd.dma_start(out=bv_b, in_=_bcast_ap(rows_d[1, :]))
        ones_r = consts.tile([1, P], BF, tag="ones_r")
        nc.vector.memset(ones_r, 1.0)
        b2k_rb = []
        for k in range(2):
            rf = consts.tile([1, D], F32, tag=f"b2r{k}f", name=f"b2r{k}f")
            nc.gpsimd.dma_start(out=rf, in_=rows_d[2 + k:3 + k, :])
            rb = consts.tile([1, D], BF, tag=f"b2r{k}b", name=f"b2r{k}b")
            nc.vector.tensor_copy(out=rb, in_=rf)
            b2k_rb.append(rb)
        if add_bo:
            bo_rf = consts.tile([1, D], F32, tag="bo_rf")
            nc.gpsimd.dma_start(out=bo_rf, in_=rows_d[4:5, :])
            bo_rb = consts.tile([1, D], BF, tag="bo_rb")
            nc.vector.tensor_copy(out=bo_rb, in_=bo_rf)

        # ---- C/D/V as callables for interleaved emission with A ----
        KT = ktp.tile([P, NC_D, TSH], BF, tag="KT")
        Vh = acts.tile([P, NV, D], BF, tag="VhH1", bufs=2, name="Vh")
        QT = ktp.tile([P, NC_D, TQ], BF, tag="QT")

        def kt_group(tch):
            for co in range(NC_D):
                ps = psA.tile([P, D], F32, tag="psA")
                for c in range(NC_D):
                    nc.tensor.matmul(
                        ps, lhsT=Ws["Wk"][:, c, co * P:(co + 1) * P],
                        rhs=KVT[:, c, tch * D:(tch + 1) * D],
                        start=(c == 0), stop=(c == NC_D - 1))
                nc.scalar.activation(
                    out=KT[:, co, tch * D:(tch + 1) * D], in_=ps,
                    func=AF.Identity, bias=bk4[:, co:co + 1])

        def v_tile(j):
            s = 104 + j * P
            ps = psA.tile([P, D], F32, tag="psA")
            for c in range(NC_D):
                nc.tensor.matmul(
                    ps, lhsT=KVT[:, c, s:s + P], rhs=Ws["Wv"][:, c, :],
                    start=(c == 0), stop=(c == NC_D - 1))
            if add_bv:
                nc.vector.tensor_tensor(out=Vh[:, j, :], in0=ps, in1=bv_b, op=OP.add)
            elif j % 2 == 0:
                nc.scalar.activation(out=Vh[:, j, :], in_=ps, func=AF.Identity)
            else:
                nc.vector.tensor_copy(out=Vh[:, j, :], in_=ps)

        def qt_group(tch):
            t0w, w = ((0, D), (D, TQ - D))[tch]
            for co in range(NC_D):
                ps = psA.tile([P, D], F32, tag="psA")
                for c in range(NC_D):
                    nc.tensor.matmul(
                        ps[:, 0:w], lhsT=Ws["Wq"][:, c, co * P:(co + 1) * P],
                        rhs=KVT[:, c, P + t0w:P + t0w + w],
                        start=(c == 0), stop=(c == NC_D - 1))
                nc.scalar.activation(
                    out=QT[:, co, t0w:t0w + w], in_=ps[:, 0:w],
                    func=AF.Identity, bias=bq4[:, co:co + 1])

        # all X tiles first (in-order PE queue: KT(0) before X(4..7) would
        # block them behind the KVT wait), then KT/V/QT as deps land
        for mt in range(NSH):
            a_tile(mt)
        kt_group(0)
        for j in range(3):
            v_tile(j)
        qt_group(0)
        warm = small.tile([P, 1], F32, tag="warm")
        nc.scalar.activation(out=warm, in_=eps_t, func=AF.Exp)

        inv_sqrt_dh = 1.0 / float(np.sqrt(DH))

        # ---- E) banded attention + fused yo/y2/stats, pipelined across lt ----
        YT = acts.tile([P, NC_D, TQ], BF, tag="YT")
        LNFT = tT_p.tile([P, NC_D, 2 * TQ], BF, tag="lnfT")
        y2 = stream_p.tile([P, 2 * NQT, D], BF, tag="y2")
        mvf_all = small.tile([P, 4 * NQT], F32, tag="mvf_all", bufs=1)

        def scores_half(lt, h0, h1, pm, den):
            """scores + additive band mask + exp/accum for heads [h0, h1)."""
            ws = lt * P - BAND
            q0 = (lt - 1) * P
            for h in range(h0, h1):
                hp, hc = 64 * (h % 2), h // 2
                ps = psB.tile([P, WIN], F32, tag="psB")
                nc.tensor.matmul(
                    ps, lhsT=QT[hp:hp + 64, hc, q0:q0 + P],
                    rhs=KT[hp:hp + 64, hc, ws:ws + WIN], start=True, stop=False)
                nc.tensor.matmul(ps, lhsT=ident, rhs=masks[:, lt - 1, :],
                                 start=False, stop=True)
                nc.scalar.activation(out=pm[:, h, :], in_=ps, func=AF.Exp,
                                     scale=inv_sqrt_dh,
                                     accum_out=den[:, h:h + 1])

        def proc_head(lt, h, pmm, r8, psy):
            """normalize+transpose+p@V for one head of tile lt."""
            pms = sm2.tile([P, WIN], BF, tag="pms", bufs=3)
            nc.vector.tensor_scalar(out=pms, in0=pmm[:, h, :],
                                    scalar1=r8[:, h:h + 1], scalar2=None,
                                    op0=OP.mult)
            ptp = psC.tile([P, 2 * P], BF, tag="psC")
            nc.tensor.transpose(ptp[:, 0:P], pms[:, 0:P], ident)
            nc.tensor.transpose(ptp[0:48, P:2 * P], pms[:, P:WIN], ident)
            pts = sm2.tile([P, 2 * P], BF, tag="pts", bufs=3)
            if sim_gelu_identity:
                # CoreSim flags the never-written ptp rows 48:128 of the
                # second chunk; HW reads them harmlessly (never consumed).
                nc.vector.tensor_copy(out=pts[:, 0:P], in_=ptp[:, 0:P])
                nc.vector.tensor_copy(out=pts[0:48, P:2 * P], in_=ptp[0:48, P:2 * P])
            elif h % 2 == 0:
                nc.vector.tensor_copy(out=pts, in_=ptp)
            else:
                # Identity is in the Exp table-set: no ACT table load here
                nc.scalar.activation(out=pts, in_=ptp, func=AF.Identity)
            hp, hc = 64 * (h % 2), h // 2
            nc.tensor.matmul(
                psy[hp:hp + 64, hc * P:(hc + 1) * P],
                lhsT=Vh[:, lt - 1, h * DH:(h + 1) * DH], rhs=pts[:, 0:P],
                start=True, stop=False)
            nc.tensor.matmul(
                psy[hp:hp + 64, hc * P:(hc + 1) * P],
                lhsT=Vh[0:48, lt, h * DH:(h + 1) * DH],
                rhs=pts[0:48, P:2 * P],
                start=False, stop=True)

        def recip_den(den):
            r8 = small.tile([P, H], F32, tag="r8", bufs=2)
            nc.vector.reciprocal(out=r8, in_=den)
            return r8

        def proc_tile(plt, ppmm, pr8, lt=None, pm=None, den=None):
            """process tile plt; optionally interleave scores for lt."""
            psy = psA.tile([P, D], F32, tag="psy", bufs=2)
            r8 = None
            for h in range(H):
                if lt is not None and 3 + h < H:
                    scores_half(lt, 3 + h, 4 + h, pm, den)
                if lt is not None and 3 + h == H - 1:
                    r8 = recip_den(den)
                proc_head(plt, h, ppmm, pr8, psy)
            nc.vector.tensor_copy(
                out=YT[:, :, (plt - 1) * P:plt * P],
                in_=psy.rearrange("p (c q) -> p c q", c=NC_D))
            return r8

        def f_yo(plt):
            """yo for tile plt; y2 = Xk + yo (k0 DVE+stats, k1 gpsimd)."""
            mt = plt - 1
            ps = psA.tile([P, D], F32, tag="psA")
            for c in range(NC_D):
                nc.tensor.matmul(
                    ps, lhsT=YT[:, c, mt * P:(mt + 1) * P], rhs=Ws["Wo"][:, c, :],
                    start=(c == 0), stop=(c == NC_D - 1 and not add_bo))
            if add_bo:
                nc.tensor.matmul(ps, lhsT=ones_r[:, 0:P], rhs=bo_rb,
                                 start=False, stop=True)
            yo = outp.tile([P, D], BF, tag="yo", bufs=2)
            nc.scalar.activation(out=yo, in_=ps, func=AF.Identity)
            for k in range(2):
                i12 = k * NQT + mt
                nc.vector.tensor_tensor(out=y2[:, i12, :], in0=yo,
                                        in1=Xk[:, i12, :], op=OP.add)
                ln_stats(y2[:, i12, :],
                         mvf_all[:, 4 * mt + 2 * k:4 * mt + 2 * k + 2])

        def ln_finish(mt):
            """LN_f normalize + transpose for both streams of tile mt."""
            vcols = mvf_all.rearrange("p (t two) -> p two t", two=2)
            rst = newton_rsqrt(vcols[:, 1, 2 * mt:2 * mt + 2], 2, "f")
            for k in range(2):
                i12 = k * NQT + mt
                negmr = small.tile([P, 1], F32, tag="negmr", bufs=4)
                nc.vector.tensor_scalar(
                    out=negmr, in0=mvf_all[:, 4 * mt + 2 * k:4 * mt + 2 * k + 1],
                    scalar1=rst[:, k:k + 1], scalar2=-1.0,
                    op0=OP.mult, op1=OP.mult)
                lnf = sm2.tile([P, D], BF, tag="lnf", bufs=2)
                nc.scalar.activation(out=lnf, in_=y2[:, i12, :],
                                     func=AF.Identity, scale=rst[:, k:k + 1],
                                     bias=negmr)
                eng = nc.sync if k == 0 else nc.scalar
                eng.dma_start_transpose(
                    out=LNFT[:, :, i12 * P:(i12 + 1) * P], in_=lnf)

        prev = None
        for lt in range(1, NQT + 1):
            pm = sm2.tile([P, H, WIN], BF, tag="pm_a", bufs=2)
            den = small.tile([P, H], F32, tag="den", bufs=2)
            scores_half(lt, 0, 3, pm, den)
            if prev is None:
                scores_half(lt, 3, H, pm, den)
                r8 = recip_den(den)
                prev = (lt, pm, r8)
                continue
            plt, ppmm, pr8 = prev
            r8 = proc_tile(plt, ppmm, pr8, lt=lt, pm=pm, den=den)
            f_yo(plt)
            ln_finish(plt - 1)
            prev = (lt, pm, r8)
            if lt == 2:
                kt_group(1)
                for j in range(3, NV):
                    v_tile(j)
            elif lt == 3:
                qt_group(1)
        plt, ppmm, pr8 = prev

        # ---- G) FFN; FFN1(tch+1) interleaved into FFN2(tch); b2k via PE ----
        def ffn1_group(tch, dh, H1g):
            ps = psA.tile([P, D], F32, tag="psA")
            for c in range(NC_D):
                nc.tensor.matmul(
                    ps, lhsT=W1s[:, c, dh * P:(dh + 1) * P],
                    rhs=LNFT[:, c, tch * D:(tch + 1) * D],
                    start=(c == 0), stop=(c == NC_D - 1))
            nc.scalar.activation(out=H1g[:, dh, :], in_=ps,
                                 func=AF_GELU, scale=1.0 / SC,
                                 bias=b1_16[:, dh:dh + 1])

        # Gelu prefetch, then the last attention tile's processing with the
        # first FFN1 groups as PE filler (LNFT 0..3 landed at ln_finish(3))
        nc.scalar.activation(out=warm, in_=eps_t, func=AF_GELU)
        H1g = acts.tile([P, 16, D], BF, tag="VhH1", bufs=2, name="H1g0")
        psy = psA.tile([P, D], F32, tag="psy", bufs=2)
        for h in range(H):
            if h >= 4:
                ffn1_group(0, h - 4, H1g)
            proc_head(plt, h, ppmm, pr8, psy)
        nc.vector.tensor_copy(
            out=YT[:, :, (plt - 1) * P:plt * P],
            in_=psy.rearrange("p (c q) -> p c q", c=NC_D))
        f_yo(plt)
        ln_finish(plt - 1)
        for dh in range(4, 16):
            ffn1_group(0, dh, H1g)
        for tch in range(3):
            nxt = None
            if tch + 1 < 3:
                nxt = acts.tile([P, 16, D], BF, tag="VhH1", bufs=2,
                                name=f"H1g{tch + 1}")
            for s0 in range(0, 4, 2):
                mvg = small.tile([P, 4], F32, tag="mvg", bufs=2)
                y3s = []
                for j in range(2):
                    mtg = tch * 4 + s0 + j
                    ps = psA.tile([P, D], F32, tag="psA")
                    for dh in range(16):
                        nc.tensor.matmul(
                            ps, lhsT=H1g[:, dh, (s0 + j) * P:(s0 + j + 1) * P],
                            rhs=W2s[:, dh, :], start=(dh == 0), stop=False)
                    nc.tensor.matmul(ps, lhsT=ones_r[:, 0:P],
                                     rhs=b2k_rb[mtg // NQT],
                                     start=False, stop=True)
                    y3 = outp.tile([P, D], F32, tag="y3", bufs=3)
                    nc.vector.tensor_tensor(out=y3, in0=ps, in1=y2[:, mtg, :],
                                            op=OP.add)
                    ln_stats(y3, mvg[:, 2 * j:2 * j + 2])
                    y3s.append(y3)
                if nxt is not None:
                    for dh in range(8 * (s0 // 2), 8 * (s0 // 2) + 8):
                        ffn1_group(tch + 1, dh, nxt)
                rstg = newton_rsqrt(
                    mvg.rearrange("p (t two) -> p two t", two=2)[:, 1, 0:2], 2, "g")
                for j in range(2):
                    mtg = tch * 4 + s0 + j
                    negmr = small.tile([P, 1], F32, tag="negmr", bufs=4)
                    nc.vector.tensor_scalar(out=negmr, in0=mvg[:, 2 * j:2 * j + 1],
                                            scalar1=rstg[:, j:j + 1], scalar2=-1.0,
                                            op0=OP.mult, op1=OP.mult)
                    o_sb = outp.tile([P, D], F32, tag="o_sb", bufs=2)
                    nc.scalar.activation(out=o_sb, in_=y3s[j], func=AF.Identity,
                                         scale=rstg[:, j:j + 1], bias=negmr)
                    nc.sync.dma_start(out=out_t[mtg], in_=o_sb)
            if nxt is not None:
                H1g = nxt

    nc.finalize()
    return nc


_PROG_CACHE = {}


def kernel(**inputs) -> np.ndarray:
    f32 = np.float32
    bf = ml_dtypes.bfloat16
    x_m = np.asarray(inputs["x_m"], f32)
    A = np.asarray(inputs["A"], f32)
    g = {kk: np.asarray(v, f32) for kk, v in inputs.items()}

    # fold LN affine params into following matmuls (exact algebra)
    Wq = g["ln_q_g"][:, None] * g["Wq"]
    bq = g["bq"] + g["ln_q_b"] @ g["Wq"]
    Wk = g["ln_kv_g"][:, None] * g["Wk"]
    bk = g["bk"] + g["ln_kv_b"] @ g["Wk"]
    Wv = g["ln_kv_g"][:, None] * g["Wv"]
    bv = g["bv"] + g["ln_kv_b"] @ g["Wv"]
    W1 = g["ln_f_g"][:, None] * g["W1"]
    b1 = g["b1"] + g["ln_f_b"] @ g["W1"]

    add_bo = bool(np.any(g["bo"] != 0.0))
    add_bin = bool(np.any(g["b_in"] != 0.0))
    add_bv = bool(np.any(bv != 0.0))
    key = (add_bo, add_bin, add_bv)
    if key not in _PROG_CACHE:
        _PROG_CACHE[key] = build_program(add_bo, add_bin=add_bin, add_bv=add_bv)
    nc = _PROG_CACHE[key]

    common = {
        "Win": np.ascontiguousarray(g["W_in"].astype(bf)),
        "Wq": np.ascontiguousarray(Wq.astype(bf)),
        "Wk": np.ascontiguousarray(Wk.astype(bf)),
        "Wv": np.ascontiguousarray(Wv.astype(bf)),
        "Wo": np.ascontiguousarray((SC * g["Wo"]).astype(bf)),
        "W1": np.ascontiguousarray((SC * W1).astype(bf)),
        "W2": np.ascontiguousarray((SC * g["W2"]).astype(bf)),
    }

    # gate: What' = 128*sigmoid(6(A-0.5))  [B, T, K]
    What = SC / (1.0 + np.exp(-6.0 * (A - 0.5)))
    # padded transposed input [B, 512, T+256]
    xmp = np.zeros((B, D, T + 2 * P), f32)
    xmp[:, :, P:P + T] = np.transpose(x_m, (0, 2, 1))

    # additive band masks per (th, lt): [128 q-part, 176 key-window]
    jj = np.arange(WIN)
    pp = np.arange(P)
    band = ((jj[None, :] >= pp[:, None]) & (jj[None, :] <= pp[:, None] + 2 * BAND))

    in_maps = []
    for c in range(8):
        b, th = c // 2, c % 2
        im = dict(common)
        im["xmT"] = np.ascontiguousarray(
            xmp[b, :, th * TQ:th * TQ + TSH].astype(bf))
        sm = np.zeros((P, 36), f32)
        for k in range(2):
            for lt in range(NQT):
                sm[:, k * NQT + lt] = What[b, th * TQ + lt * P:th * TQ + (lt + 1) * P, k]
        sm[:, 12:16] = bq.reshape(4, P).T
        sm[:, 16:20] = bk.reshape(4, P).T
        sm[:, 20:36] = b1.reshape(16, P).T
        im["smalls"] = sm
        rows = np.stack([g["b_in"], bv,
                         SC * (g["b2"] + g["spk_tags"][0]),
                         SC * (g["b2"] + g["spk_tags"][1]),
                         SC * g["bo"]])
        im["rows"] = rows.astype(f32)
        mk = np.zeros((P, NQT, WIN), f32)
        for lt in range(1, NQT + 1):
            ws_true = th * TQ + lt * P - BAND - P  # true T coord of window col 0
            valid = (jj[None, :] + ws_true >= 0) & (jj[None, :] + ws_true < T)
            mk[:, lt - 1, :] = np.where(band & valid, 0.0, -1e30)
        im["masks"] = np.ascontiguousarray(mk.reshape(P, NQT * WIN).astype(bf))
        in_maps.append(im)

    res = run_bass_kernel_spmd(nc, in_maps, core_ids=list(range(8)))
    out = np.zeros((B, KSP * T, D), f32)
    gs, bs = g["ln_s_g"], g["ln_s_b"]
    for c in range(8):
        b, th = c // 2, c % 2
        r = res.results[c]["out"]
        for k in range(2):
            out[b, k * T + th * TQ:k * T + (th + 1) * TQ] = \
                r[k * TQ:(k + 1) * TQ] * gs + bs
    return out
